# revision 2
# baseline (speedup 1.0000x reference)
"""Trainium2 Bass kernel for 5x5 patch extraction — v7 (PE-shift, fast fill).

Full input:  images [8, 128, 128, 32] f32
Full output: [8, 128, 128, 800] f32 where
  out[b, i, j, ki*160 + kj*32 + c] = images_padded[b, i+ki, j+kj, c]
  (spatial zero-padding of 2 on each side).

Sharding: data-parallel over batch; core b handles image b.

Design: load the image ONCE (2MB) into img5's center slot; build the
four row-shifted copies on-chip with the otherwise-idle TensorE
(shift-matrix fp32 matmuls into PSUM, bank = shift), copied PSUM->SBUF
by the otherwise-idle ScalarE. Border rows (zero padding) fall out of
the zero columns of the shift matrices. DVE then builds contiguous
800-float records per j-chunk (triple buffered) and sync DMAs them out
with 25.6KB/partition descriptors. DMA engine traffic is 54.7MB vs
62.9MB for a 5x-amplified-load design; the 16 shared per-core SDMA
engines (~22-26GB/s each, ~355-425GB/s aggregate) carry every byte.

Fill-path tuning (from instruction traces):
- input arrives PIECE-CONTIGUOUS in DRAM (host packs column pieces of
  256/768/1536/1536 data cols) -- descriptor generation for DMAs
  executing in the first ~10us runs ~20x slower for strided sources;
- shm (shift matrices) loads FIRST on the sync queue; piece0 + piece1
  load first on the scalar queue, so the two critical loads crawl in
  parallel; pieces 2/3 issue later between PSUM copies;
- TensorE runs 10 throwaway matmuls on a never-written scratch tile
  before the real work to ramp the PE out of its low p-state (cold
  fp32 matmuls run ~2x slower);
- head chunks are tiny (2,2,4 cols) so the first output DMA launches
  as soon as one 256-col tile of all four shifts lands; tail chunks
  taper (4,4,4,2,2) to shrink the drain;
- staging gates per 256-col TILE (s_cp[3] >= tiles), not per piece.

Hardware findings baked in (measured on TRN2):
- HWDGE splits one DMA across n = (largest divisor of outer AP count
  <= 16) SDMA engines; odd outer counts pin to ONE engine.
- All dynamic-DMA queues (sync/scalar/gpsimd) share the SAME 16
  engines; multi-queue adds no bandwidth, only issue parallelism.
- <= 1 outstanding DMA per semaphore; <= 32 DMA semaphores.
"""

from contextlib import ExitStack

import numpy as np

import concourse.bass as bass
import concourse.bacc as bacc
import concourse.mybir as mybir
from concourse.bass_utils import run_bass_kernel_spmd

K = 5
H = W = 128
C = 32
B = 8
PAD = (K - 1) // 2  # 2
KC = K * C  # 160
ROW = W * C  # 4096
TROW = (W + 2 * PAD) * C  # 4224
PADC = PAD * C  # 64 col-pad elems each side
CHUNKS = (
    [(0, 2), (2, 2), (4, 4)]
    + [(8 + 8 * i, 8) for i in range(13)]
    + [(112, 4), (116, 4), (120, 4), (124, 2), (126, 2)]
)
NQ = len(CHUNKS)  # 21
REC = K * K * C  # 800
STG = 8 * REC  # staged elems per partition per chunk buffer (max jc=8)
TILE = 256  # matmul moving free size
NT = ROW // TILE  # 16 col tiles
PIECES = [(0, 256), (256, 768), (1024, 1536), (2560, 1536)]  # data-col ranges
NPIECE = len(PIECES)
PIECE_OFF = [128 * sum(w for _, w in PIECES[:P]) for P in range(len(PIECES))]
SHIFTS = (-2, -1, 1, 2)  # slot ki = shift + 2 (ki=2 is the center load)
SLOT_OF_SHIFT = (0, 1, 3, 4)
NSD = 6  # round-robin write-DMA semaphores
NSV = 4  # round-robin staging semaphores
NWARM = 3  # PE p-state warmup matmuls

_NC_CACHE = {}


def _piece_of_tile(tile):
    col = tile * TILE
    for P, (c0, w) in enumerate(PIECES):
        if c0 <= col < c0 + w:
            return P
    raise AssertionError(tile)


def _tiles_for_chunk(q):
    j0, jc = CHUNKS[q]
    hi = (j0 + jc) * C + KC - PADC  # exclusive data-col bound
    return min(NT, max(1, -(-hi // TILE)))


def _build_nc():
    nc = bacc.Bacc("TRN2", target_bir_lowering=False, debug=False)
    images = nc.dram_tensor(
        "images", [H, ROW], mybir.dt.float32, kind="ExternalInput"
    )
    shifts = nc.dram_tensor(
        "shifts", [128, 4 * 128], mybir.dt.float32, kind="ExternalInput"
    )
    out = nc.dram_tensor(
        "out", [H, W, REC], mybir.dt.float32, kind="ExternalOutput"
    )

    with ExitStack() as stack:
        img5 = stack.enter_context(
            nc.sbuf_tensor("img5", [128, K * TROW], mybir.dt.float32)
        )
        shm = stack.enter_context(
            nc.sbuf_tensor("shm", [128, 4 * 128], mybir.dt.float32)
        )
        dmy = stack.enter_context(
            nc.sbuf_tensor("dmy", [128, TILE], mybir.dt.float32)
        )
        stg = [
            stack.enter_context(
                nc.sbuf_tensor(f"stg{b}", [128, STG], mybir.dt.float32)
            )
            for b in range(3)
        ]
        psum = [
            stack.enter_context(
                nc.psum_tensor(f"ps{s}", [128, TILE], mybir.dt.float32)
            )
            for s in range(4)
        ]
        pswm = stack.enter_context(
            nc.psum_tensor("pswm", [128, TILE], mybir.dt.float32)
        )
        s_ms = stack.enter_context(nc.semaphore("s_ms"))
        s_shm = stack.enter_context(nc.semaphore("s_shm"))
        s_load = [
            stack.enter_context(nc.semaphore(f"s_load{t}")) for t in range(NPIECE)
        ]
        s_mm = [stack.enter_context(nc.semaphore(f"s_mm{s}")) for s in range(4)]
        s_cp = [stack.enter_context(nc.semaphore(f"s_cp{s}")) for s in range(4)]
        sv = [stack.enter_context(nc.semaphore(f"sv{i}")) for i in range(NSV)]
        sd = [stack.enter_context(nc.semaphore(f"sd{i}")) for i in range(NSD)]

        with nc.Block() as block:
            b5 = img5[:, :]
            p5 = b5.ap[0][0]
            bshm = shm[:, :]
            pshm = bshm.ap[0][0]
            bdmy = dmy[:, :]
            pdmy = bdmy.ap[0][0]
            bs = [t[:, :] for t in stg]
            ps = [b.ap[0][0] for b in bs]
            bpsum = [t[:, :] for t in psum]
            bpswm = pswm[:, :]

            @block.tensor
            def _(tensor):
                # ramp the PE p-state on throwaway data (never-written
                # scratch; real matmuls below use start=True so PSUM
                # contents never leak)
                dlhs = bass.AP(
                    bdmy.tensor, bdmy.offset, [[pdmy, 128], [1, 128]]
                )
                for _w in range(NWARM):
                    tensor.matmul(bpswm, dlhs, bdmy, start=True, stop=True)
                tensor.wait_ge(s_shm, 16)
                for tile in range(NT):
                    P = _piece_of_tile(tile)
                    if tile == 0 or _piece_of_tile(tile - 1) != P:
                        tensor.wait_ge(s_load[P], 16)
                    col = PADC + tile * TILE
                    for s in range(4):
                        if tile >= 1:
                            tensor.wait_ge(s_cp[s], tile)
                        lhsT = bass.AP(
                            bshm.tensor,
                            bshm.offset + s * 128,
                            [[pshm, 128], [1, 128]],
                        )
                        rhs = bass.AP(
                            b5.tensor,
                            b5.offset + 2 * TROW + col,
                            [[p5, 128], [1, TILE]],
                        )
                        tensor.matmul(
                            bpsum[s], lhsT, rhs, start=True, stop=True
                        ).then_inc(s_mm[s], 1)

            @block.scalar
            def _(scalar):
                scalar.dma_start(
                    bass.AP(bshm.tensor, bshm.offset, [[pshm, 128], [1, 512]]),
                    bass.AP(shifts, 0, [[512, 128], [1, 512]]),
                ).then_inc(s_shm, 16)
                for P in (1, 2):
                    c0, w = PIECES[P]
                    scalar.dma_start(
                        bass.AP(
                            b5.tensor,
                            b5.offset + 2 * TROW + PADC + c0,
                            [[p5, 128], [1, w]],
                        ),
                        bass.AP(images, PIECE_OFF[P], [[w, 128], [1, w]]),
                    ).then_inc(s_load[P], 16)
                for tile in range(NT):
                    col = PADC + tile * TILE
                    for s in range(4):
                        scalar.wait_ge(s_mm[s], tile + 1)
                        dst = bass.AP(
                            b5.tensor,
                            b5.offset + SLOT_OF_SHIFT[s] * TROW + col,
                            [[p5, 128], [1, TILE]],
                        )
                        scalar.copy(dst, bpsum[s]).then_inc(s_cp[s], 1)
                    if tile == 0:
                        P = 3
                        c0, w = PIECES[P]
                        scalar.dma_start(
                            bass.AP(
                                b5.tensor,
                                b5.offset + 2 * TROW + PADC + c0,
                                [[p5, 128], [1, w]],
                            ),
                            bass.AP(images, PIECE_OFF[P], [[w, 128], [1, w]]),
                        ).then_inc(s_load[P], 16)

            @block.vector
            def _(vector):
                vector.memset(
                    bass.AP(
                        b5.tensor, b5.offset, [[p5, 128], [TROW, K], [1, PADC]]
                    ),
                    0.0,
                ).then_inc(s_ms, 1)
                vector.memset(
                    bass.AP(
                        b5.tensor,
                        b5.offset + TROW - PADC,
                        [[p5, 128], [TROW, K], [1, PADC]],
                    ),
                    0.0,
                ).then_inc(s_ms, 1)
                for q in range(NQ):
                    vector.wait_ge(s_cp[3], _tiles_for_chunk(q))
                    if q >= 3:
                        qq = q - 3
                        vector.wait_ge(sd[qq % NSD], 16 * (qq // NSD + 1))
                    buf = q % 3
                    j0, jc = CHUNKS[q]
                    for ki in range(K):
                        src = bass.AP(
                            b5.tensor,
                            b5.offset + ki * TROW + j0 * C,
                            [[p5, 128], [C, jc], [1, KC]],
                        )
                        dst = bass.AP(
                            bs[buf].tensor,
                            bs[buf].offset + ki * KC,
                            [[ps[buf], 128], [REC, jc], [1, KC]],
                        )
                        ins = vector.tensor_copy(dst, src)
                        if ki == K - 1:
                            ins.then_inc(sv[q % NSV], 1)

            @block.sync
            def _(sync):
                c0, w = PIECES[0]
                sync.dma_start(
                    bass.AP(
                        b5.tensor,
                        b5.offset + 2 * TROW + PADC + c0,
                        [[p5, 128], [1, w]],
                    ),
                    bass.AP(images, PIECE_OFF[0], [[w, 128], [1, w]]),
                ).then_inc(s_load[0], 16)
                for q in range(NQ):
                    buf = q % 3
                    j0, jc = CHUNKS[q]
                    sync.wait_ge(sv[q % NSV], q // NSV + 1)
                    if q >= NSD:
                        sync.wait_ge(sd[q % NSD], 16 * (q // NSD))
                    src = bass.AP(
                        bs[buf].tensor,
                        bs[buf].offset,
                        [[ps[buf], 128], [REC, jc], [1, REC]],
                    )
                    dstd = bass.AP(
                        out, j0 * REC, [[W * REC, 128], [REC, jc], [1, REC]]
                    )
                    sync.dma_start(dstd, src).then_inc(sd[q % NSD], 16)
                for i in range(NSD):
                    uses = (NQ - i + NSD - 1) // NSD
                    sync.wait_ge(sd[i], 16 * uses)

    nc.compile()
    return nc


def _get_nc():
    if "nc" not in _NC_CACHE:
        _NC_CACHE["nc"] = _build_nc()
    return _NC_CACHE["nc"]


def _shift_matrices() -> np.ndarray:
    smat = np.zeros((128, 4 * 128), dtype=np.float32)
    for si, s in enumerate(SHIFTS):
        for m in range(128):
            k = m + s
            if 0 <= k < 128:
                smat[k, si * 128 + m] = 1.0
    return smat


def run(images: np.ndarray, trace: bool = False, tmpdir=None):
    """Run on 8 cores. Returns (output [8,128,128,800], BassKernelResults)."""
    images = np.ascontiguousarray(np.asarray(images, dtype=np.float32))
    assert images.shape == (B, H, W, C), images.shape
    nc = _get_nc()
    smat = _shift_matrices()

    def _pack(img2d):
        return np.concatenate(
            [
                np.ascontiguousarray(img2d[:, c0 : c0 + w]).ravel()
                for c0, w in PIECES
            ]
        ).reshape(H, ROW)

    in_maps = [
        {"images": _pack(images[b].reshape(H, ROW)), "shifts": smat}
        for b in range(B)
    ]
    last_err = None
    for attempt in range(3):
        try:
            res = run_bass_kernel_spmd(
                nc, in_maps, core_ids=list(range(B)), trace=trace, tmpdir=tmpdir
            )
            break
        except Exception as e:  # transient NRT device errors observed rarely
            last_err = e
            import time as _time

            _time.sleep(2.0 * (attempt + 1))
    else:
        raise last_err
    out = np.stack([res.results[b]["out"] for b in range(B)], axis=0)
    return out.reshape(B, H, W, REC), res


def kernel(images: np.ndarray) -> np.ndarray:
    out, _ = run(images)
    return out
